# revision 14
# baseline (speedup 1.0000x reference)
"""Trainium2 Bass kernel: LayerNorm + biased multi-head attention + projection.

Shapes (full problem): x [16, 1024, 2048], H=16 heads, head dim 128,
qkv_w [2048, 6144], proj_w [2048, 2048], attention_biases [16, 1024],
bias_idxs [1024, 1024] int32.

Strategy: pure data-parallel over batch across the 8 NeuronCores
(2 batch elements per core); no collectives.  All matmuls bf16 operands
with fp32 PSUM accumulation.

Per core, one fused stream:

  Phase A) LayerNorm (bn_stats) on bf16 x, PE-transpose to a dim-major
     layout, QKV projections.  Q/K produced transposed ([head_dim,
     tokens]) with the attention scale folded into Wq; V in natural
     layout; staged in DRAM scratch.  Batch 1's LayerNorm is interleaved
     into batch 0's QK matmul stream.
  Slots) attention per (batch, head), batch-outer so batch 0's output
     projection can interleave into batch 1's head slots:
       - scores S^T = K-tile x Q^T built transposed ([keys, queries]) in
         [128, 1024] two-bank PSUM tiles, software-pipelined one key-tile
         ahead so the exp/bias chain never starves the PE;
       - P^T = exp(S^T) (one ACT op per key-tile) * exp(bias)^T (one DVE
         multiply per key-tile).  The [H, N, N] bias matrix is
         block-Toeplitz in 128-token blocks (bias depends only on
         query-block minus key-block), so the host ships only 15 unique
         [128, 128] exp(bias) blocks per head and the per-key-tile bias
         operand is a contiguous column slice of that buffer;
       - PV and the softmax denominator: PV accumulates on the PE; the
         denominator is a DVE add-tree over the P^T tiles plus one GPSIMD
         partition_all_reduce (replicated across partitions), which keeps
         512 ones-column matmuls off the PE and frees two PSUM banks;
       - O^T is copied raw out of PSUM (freeing the PSUM bank for the
         next slot) and the reciprocal+normalize+store tail is emitted
         inside the NEXT slot's stream so it never head-of-line blocks
         the DVE queue.
  Proj) output projection per (token-tile, dim-block) chunk, interleaved
     2-3 chunks per head slot of the following batch; the last batch's
     projection drains at the end.

Host-side preprocessing (parameter folding only): ln_gamma folded into
qkv_w, ln_beta/qkv_b folded into a qkv bias vector, attention scale
folded into Wq, weights cast to bf16 and pre-tiled, x cast to bf16, and
the attention-bias table gathered into the 15-block exp(bias) form.
"""

import numpy as np
from contextlib import ExitStack

import ml_dtypes

import concourse.bass as bass
import concourse.bacc as bacc
import concourse.tile as tile
import concourse.mybir as mybir
import concourse.bass_isa as bass_isa
from concourse.alu_op_type import AluOpType
from concourse.bass_utils import run_bass_kernel_spmd
from concourse.masks import make_identity

F32 = mybir.dt.float32
BF16 = mybir.dt.bfloat16
AF = mybir.ActivationFunctionType
P = 128
N_CORES = 8


def build_program(B_local, N, DIM, H, eps=1e-5, qkv_bias=False, proj_bias=False,
                  trn_type="TRN2", phases=("A", "B", "C")):
    D = P                      # per-head dim
    DH = H * D                 # 2048
    KT = DIM // P              # 16 contraction tiles
    NT = N // P                # 8 token/key tiles
    CB = 512                   # psum column block (1 bank of f32)
    NCB = N // CB              # 2
    VB = 512
    NVB = DH // VB             # 4
    EB = 512
    NEB = DIM // EB            # 4
    HC = DH // P               # 16 head chunks
    NBLK = 2 * NT - 1          # 15 unique bias blocks
    QKOC = 2 * HC              # 32 q+k output chunks
    assert DIM % 512 == 0
    SG = DIM // 512            # bn_stats subgroups
    reps = list(phases).count("A")

    nc = bacc.Bacc(trn_type, target_bir_lowering=False, debug=False)

    x_d = nc.dram_tensor("x", [B_local, N, DIM], BF16, kind="ExternalInput").ap()
    wqk_d = nc.dram_tensor("wqk", [QKOC, P, KT, P], BF16,
                           kind="ExternalInput").ap()
    wv_d = nc.dram_tensor("wv", [NVB, P, KT, VB], BF16,
                          kind="ExternalInput").ap()
    wp_d = nc.dram_tensor("wp", [DH, DIM], BF16, kind="ExternalInput").ap()
    bblk_d = nc.dram_tensor("biasblk", [H, P, NBLK * P], BF16,
                            kind="ExternalInput").ap()
    qbqk_d = qbv_d = pb_d = None
    if qkv_bias:
        qbqk_d = nc.dram_tensor("qb_qk", [2 * DH], F32, kind="ExternalInput").ap()
        qbv_d = nc.dram_tensor("qb_v", [DH], F32, kind="ExternalInput").ap()
    if proj_bias:
        pb_d = nc.dram_tensor("pb", [DIM], F32, kind="ExternalInput").ap()
    out_d = nc.dram_tensor("out", [B_local, N, DIM], F32, kind="ExternalOutput").ap()

    with tile.TileContext(nc) as tc:
        with ExitStack() as top:
            dram = top.enter_context(tc.tile_pool(name="dram", bufs=1, space="DRAM"))
            qkT_s = dram.tile([B_local, 2 * DH, N], BF16, tag="qkT", name="qkT")
            vnat_s = dram.tile([B_local, N, DH], BF16, tag="vnat", name="vnat")
            oT_s = dram.tile([B_local, H, D, N], BF16, tag="oT", name="oT")

            const = top.enter_context(tc.tile_pool(name="const", bufs=1))
            ident = const.tile([P, P], BF16, tag="ident", name="ident")
            make_identity(nc, ident)
            eps_t = const.tile([P, 1], F32, tag="eps", name="eps_t")
            nc.gpsimd.memset(eps_t, eps)
            if qkv_bias:
                qbqk_sb = const.tile([P, QKOC], F32, tag="qbqk", name="qbqk_sb")
                nc.sync.dma_start(out=qbqk_sb,
                                  in_=qbqk_d.rearrange("(oc p) -> p oc", p=P))
                qbv_row = const.tile([1, DH], F32, tag="qbvr", name="qbv_row")
                nc.sync.dma_start(out=qbv_row,
                                  in_=qbv_d.rearrange("(a d) -> a d", a=1))
                qbv_bc = const.tile([P, DH], F32, tag="qbvb", name="qbv_bc")
                nc.gpsimd.partition_broadcast(qbv_bc, qbv_row)
            if proj_bias:
                pb_row = const.tile([1, DIM], F32, tag="pbr", name="pb_row")
                nc.sync.dma_start(out=pb_row,
                                  in_=pb_d.rearrange("(a d) -> a d", a=1))
                pb_bc = const.tile([P, DIM], F32, tag="pbb", name="pb_bc")
                nc.gpsimd.partition_broadcast(pb_bc, pb_row)

            # persistent attention pools (prefetch across slots)
            bpool = top.enter_context(tc.tile_pool(name="biasb", bufs=2))
            qpool = top.enter_context(tc.tile_pool(name="qb", bufs=2))
            kpool = top.enter_context(tc.tile_pool(name="kb", bufs=2))
            vpool = top.enter_context(tc.tile_pool(name="vb", bufs=2))
            tpool = top.enter_context(tc.tile_pool(name="tb", bufs=3))
            ppool = top.enter_context(tc.tile_pool(name="pb", bufs=6))
            treep = top.enter_context(tc.tile_pool(name="tree", bufs=1))
            gpool = top.enter_context(tc.tile_pool(name="gb", bufs=2))
            drpool = top.enter_context(tc.tile_pool(name="drb", bufs=2))
            rrpool = top.enter_context(tc.tile_pool(name="rrb", bufs=2))
            orawp = top.enter_context(tc.tile_pool(name="oraw", bufs=2))
            ospool = top.enter_context(tc.tile_pool(name="osb", bufs=2))
            # psum used by QKV chains in phase A and proj chunks later
            fillps = top.enter_context(
                tc.tile_pool(name="fillps", bufs=2, space="PSUM"))

            def preload(b, h):
                bias_sb = bpool.tile([P, NBLK * P], BF16, tag="bias",
                                     name="bias_sb")
                nc.sync.dma_start(out=bias_sb, in_=bblk_d[h])
                q_sb = qpool.tile([P, N], BF16, tag="q", name="q_sb")
                nc.sync.dma_start(out=q_sb, in_=qkT_s[b, h * P:(h + 1) * P, :])
                k_sb = kpool.tile([P, N], BF16, tag="k", name="k_sb")
                nc.sync.dma_start(
                    out=k_sb, in_=qkT_s[b, DH + h * P:DH + (h + 1) * P, :])
                v_sb = vpool.tile([P, NT, P], BF16, tag="v", name="v_sb")
                nc.sync.dma_start(
                    out=v_sb,
                    in_=vnat_s[b, :, h * P:(h + 1) * P].rearrange(
                        "(jc p) d -> p jc d", p=P))
                return bias_sb, q_sb, k_sb, v_sb

            for _rep in range(reps):
                # ---------------- Phase A: LN + QKV ----------------
                with ExitStack() as actx:
                    xall = actx.enter_context(tc.tile_pool(name="xall", bufs=1))
                    xpool = actx.enter_context(tc.tile_pool(name="xa", bufs=2))
                    xcpool = actx.enter_context(tc.tile_pool(name="xca", bufs=2))
                    stats = actx.enter_context(tc.tile_pool(name="stats", bufs=2))
                    wpool = actx.enter_context(tc.tile_pool(name="wa", bufs=2))
                    wvpool = actx.enter_context(tc.tile_pool(name="wva", bufs=2))
                    evq = actx.enter_context(tc.tile_pool(name="evq", bufs=2))
                    evv = actx.enter_context(tc.tile_pool(name="evv", bufs=3))
                    tpsum = actx.enter_context(
                        tc.tile_pool(name="tpsA", bufs=2, space="PSUM"))

                    xc_all = xall.tile([P, B_local * KT, N], BF16, tag="xc_all",
                                       name="xc_all")

                    def emit_ln(b, tt):
                        x_t = xpool.tile([P, DIM], BF16, tag="x_t", name="x_t")
                        nc.sync.dma_start(out=x_t,
                                          in_=x_d[b, tt * P:(tt + 1) * P, :])
                        st = stats.tile([P, SG, 6], F32, tag="st", name="st")
                        for sg in range(SG):
                            nc.vector.bn_stats(
                                out=st[:, sg, :],
                                in_=x_t[:, sg * 512:(sg + 1) * 512])
                        sv = stats.tile([P, 8], F32, tag="sv", name="sv")
                        mv, sd, rstd, nmu, nmr = (sv[:, 0:2], sv[:, 2:3],
                                                  sv[:, 3:4], sv[:, 4:5],
                                                  sv[:, 5:6])
                        nc.vector.bn_aggr(out=mv, in_=st)
                        nc.scalar.activation(sd, mv[:, 1:2], AF.Sqrt,
                                             bias=eps_t, scale=1.0)
                        nc.vector.reciprocal(rstd, sd)
                        nc.vector.tensor_scalar_mul(nmu, mv[:, 0:1], -1.0)
                        nc.vector.tensor_tensor(nmr, nmu, rstd, AluOpType.mult)
                        xc_t = xcpool.tile([P, DIM], BF16, tag="xc_t",
                                           name="xc_t")
                        nc.vector.tensor_scalar(xc_t, x_t, rstd, nmr,
                                                AluOpType.mult, AluOpType.add)
                        for kc in range(KT):
                            tp = tpsum.tile([P, P], BF16, tag="tp", name="tp")
                            nc.tensor.transpose(
                                tp, xc_t[:, kc * P:(kc + 1) * P], ident)
                            nc.scalar.copy(
                                xc_all[:, b * KT + kc, tt * P:(tt + 1) * P], tp)

                    def emit_qk(b, oc):
                        w_t = wpool.tile([P, KT, P], BF16, tag="w_t",
                                         name="w_t")
                        nc.sync.dma_start(out=w_t, in_=wqk_d[oc])
                        ev = evq.tile([P, N], BF16, tag="ev", name="ev")
                        for cb in range(NCB):
                            ps = fillps.tile([P, CB], F32, tag="fill",
                                             name="ps")
                            for kc in range(KT):
                                nc.tensor.matmul(
                                    ps, (w_t[:, kc, :]),
                                    (xc_all[:, b * KT + kc,
                                            cb * CB:(cb + 1) * CB]),
                                    start=(kc == 0), stop=(kc == KT - 1))
                            # evacuate on ACT: the DVE queue carries the
                            # LayerNorm bursts, which would head-of-line
                            # block these copies and stall the PE on psum
                            if qkv_bias:
                                nc.scalar.activation(
                                    ev[:, cb * CB:(cb + 1) * CB], ps,
                                    AF.Copy, scale=1.0)
                                nc.vector.tensor_scalar_add(
                                    ev[:, cb * CB:(cb + 1) * CB],
                                    ev[:, cb * CB:(cb + 1) * CB],
                                    qbqk_sb[:, oc:oc + 1])
                            else:
                                nc.scalar.copy(
                                    ev[:, cb * CB:(cb + 1) * CB], ps)
                        nc.sync.dma_start(
                            out=qkT_s[b, oc * P:(oc + 1) * P, :], in_=ev)

                    def emit_v(b, vg):
                        wv_t = wvpool.tile([P, KT, VB], BF16, tag="wv_t",
                                           name="wv_t")
                        nc.sync.dma_start(out=wv_t, in_=wv_d[vg])
                        for tt in range(NT):
                            ps = fillps.tile([P, CB], F32, tag="fill",
                                             name="ps")
                            for kc in range(KT):
                                nc.tensor.matmul(
                                    ps[:, :VB],
                                    (xc_all[:, b * KT + kc,
                                            tt * P:(tt + 1) * P]),
                                    (wv_t[:, kc, :]),
                                    start=(kc == 0), stop=(kc == KT - 1))
                            ev = evv.tile([P, VB], BF16, tag="evv", name="ev")
                            if qkv_bias:
                                nc.scalar.copy(ev, ps[:, :VB])
                                nc.vector.tensor_tensor(
                                    ev, ev,
                                    qbv_bc[:, vg * VB:(vg + 1) * VB],
                                    AluOpType.add)
                            else:
                                nc.scalar.copy(ev, ps[:, :VB])
                            nc.scalar.dma_start(
                                out=vnat_s[b, tt * P:(tt + 1) * P,
                                           vg * VB:(vg + 1) * VB],
                                in_=ev)

                    for tt in range(NT):
                        emit_ln(0, tt)
                    nxt = list(range(NT)) if B_local > 1 else []
                    for oc in range(QKOC):
                        emit_qk(0, oc)
                        if nxt and oc % 4 == 2:
                            emit_ln(1, nxt.pop(0))
                    for tt in nxt:
                        emit_ln(1, tt)
                    for vg in range(NVB):
                        emit_v(0, vg)
                    for b in range(1, B_local):
                        for oc in range(QKOC):
                            emit_qk(b, oc)
                        for vg in range(NVB):
                            emit_v(b, vg)

                # ------------- Slots: attention + projection -------------
                if "B" not in phases:
                    continue
                do_proj = "C" in phases
                with ExitStack() as bctx:
                    spools = [
                        bctx.enter_context(
                            tc.tile_pool(name=f"sps{i}", bufs=1, space="PSUM"))
                        for i in range(2)]
                    opsum = bctx.enter_context(
                        tc.tile_pool(name="ops", bufs=1, space="PSUM"))
                    pending_drains = []
                    if do_proj:
                        wppool = bctx.enter_context(
                            tc.tile_pool(name="wpc", bufs=1))
                        ocpool = bctx.enter_context(
                            tc.tile_pool(name="occ", bufs=2))
                        outpool = bctx.enter_context(
                            tc.tile_pool(name="outc", bufs=4))
                        wp_sb = wppool.tile([P, HC, DIM], BF16, tag="wp",
                                            name="wp_sb")
                        nc.scalar.dma_start(
                            out=wp_sb,
                            in_=wp_d.rearrange("(hc p) e -> p hc e", p=P))

                        def make_proj_items(bb):
                            items = []
                            oc_hold = {}
                            for tt in range(NT):
                                def load_oc(tt=tt):
                                    o_c = ocpool.tile([P, HC, P], BF16,
                                                      tag="oc", name="o_c")
                                    nc.scalar.dma_start(
                                        out=o_c,
                                        in_=oT_s[bb, :, :,
                                                 tt * P:(tt + 1) * P
                                                 ].rearrange("h d t -> d h t"))
                                    oc_hold[tt] = o_c
                                items.append(load_oc)
                                for eg in range(NEB):
                                    def chunk(tt=tt, eg=eg):
                                        o_c = oc_hold[tt]
                                        ps = fillps.tile([P, CB], F32,
                                                         tag="fill", name="ps")
                                        for hc in range(HC):
                                            nc.tensor.matmul(
                                                ps, (o_c[:, hc, :]),
                                                (wp_sb[:, hc,
                                                       eg * EB:(eg + 1) * EB]),
                                                start=(hc == 0),
                                                stop=(hc == HC - 1))

                                        # evacuation is deferred to the next
                                        # pull so its wait-on-PE never
                                        # head-of-line blocks the DVE queue
                                        def drain(ps=ps, tt=tt, eg=eg):
                                            oe = outpool.tile([P, EB], F32,
                                                              tag="oe",
                                                              name="oe")
                                            if proj_bias:
                                                nc.vector.tensor_tensor(
                                                    oe, ps,
                                                    pb_bc[:,
                                                          eg * EB:(eg + 1) * EB],
                                                    AluOpType.add)
                                            else:
                                                nc.vector.tensor_copy(oe, ps)
                                            nc.scalar.dma_start(
                                                out=out_d[bb,
                                                          tt * P:(tt + 1) * P,
                                                          eg * EB:(eg + 1) * EB],
                                                in_=oe)
                                        pending_drains.append(drain)
                                    items.append(chunk)
                            return items

                    def attention_slot(b, h, tl, fill_items, prev_tail):
                        bias_sb, q_sb, k_sb, v_sb = tl
                        o_ps = opsum.tile([P, N], F32, tag="o", name="o_ps")

                        def pull(n=1):
                            for _ in range(n):
                                while pending_drains:
                                    pending_drains.pop(0)()
                                if fill_items:
                                    fill_items.pop(0)()

                        s_tiles = {}

                        def S(jc):
                            s_ps = spools[jc % 2].tile([P, N], F32, tag="s",
                                                       name="s_ps")
                            for ic in range(NCB):
                                nc.tensor.matmul(
                                    s_ps[:, ic * CB:(ic + 1) * CB],
                                    (k_sb[:, jc * P:(jc + 1) * P]),
                                    (q_sb[:, ic * CB:(ic + 1) * CB]),
                                    start=True, stop=True)
                            s_tiles[jc] = s_ps

                        # score LADDER with PVs lagged LAG key-tiles behind:
                        # the PE pays the S->exp->mult chain latency once per
                        # slot instead of once per key-tile (in-order engine)
                        LAG = 4
                        pull()
                        p_tiles = {}
                        e_tiles = {}
                        f_tiles = {}
                        for u in range(NT + LAG):
                            jc = u
                            if jc < NT:
                                S(jc)
                                t_sb = tpool.tile([P, N], BF16, tag="t",
                                                  name="t_sb")
                                nc.scalar.activation(t_sb, s_tiles.pop(jc),
                                                     AF.Exp)
                                p_sb = ppool.tile([P, N], BF16, tag="p",
                                                  name="p_sb")
                                off = (NT - 1 - jc) * P
                                nc.vector.tensor_tensor(
                                    p_sb, t_sb, bias_sb[:, off:off + N],
                                    AluOpType.mult)
                                p_tiles[jc] = p_sb
                                if jc % 2 == 1:
                                    k2 = jc // 2
                                    e = treep.tile([P, N], BF16, tag=f"e{k2}",
                                                   name="e")
                                    nc.vector.tensor_tensor(
                                        e, p_tiles[jc - 1], p_tiles[jc],
                                        AluOpType.add)
                                    e_tiles[k2] = e
                                if jc == 3:
                                    f0 = treep.tile([P, N], BF16, tag="f0",
                                                    name="f0")
                                    nc.vector.tensor_tensor(
                                        f0, e_tiles.pop(0), e_tiles.pop(1),
                                        AluOpType.add)
                                    f_tiles[0] = f0
                            pj = u - LAG
                            if 0 <= pj:
                                pv = p_tiles.pop(pj)
                                for ic in range(NCB):
                                    nc.tensor.matmul(
                                        o_ps[:, ic * CB:(ic + 1) * CB],
                                        (v_sb[:, pj, :]),
                                        (pv[:, ic * CB:(ic + 1) * CB]),
                                        start=(pj == 0), stop=(pj == NT - 1))
                            if u == 5:
                                pull()
                            if u == 4 and prev_tail is not None:
                                prev_tail()
                                prev_tail = None
                            if u == 9:
                                pull()
                        if prev_tail is not None:
                            prev_tail()
                        f1 = treep.tile([P, N], BF16, tag="f1", name="f1")
                        nc.vector.tensor_tensor(f1, e_tiles.pop(2),
                                                e_tiles.pop(3), AluOpType.add)
                        g = gpool.tile([P, N], BF16, tag="g", name="g")
                        nc.vector.tensor_tensor(g, f_tiles.pop(0), f1,
                                                AluOpType.add)
                        dr = drpool.tile([P, N], F32, tag="dr", name="dr")
                        nc.gpsimd.partition_all_reduce(dr, g, P,
                                                       bass_isa.ReduceOp.add)
                        oraw = orawp.tile([P, N], BF16, tag="or", name="oraw")
                        nc.vector.tensor_copy(oraw, o_ps)

                        def tail():
                            rr = rrpool.tile([P, N], BF16, tag="rr", name="rr")
                            # 1/d at bf16: ~0.4% relative error on a pure
                            # normalization factor, far inside the 2e-2 gate
                            with nc.allow_low_precision(
                                    reason="softmax 1/denominator in bf16"):
                                nc.vector.reciprocal(rr, dr)
                            o_sb = ospool.tile([P, N], BF16, tag="os",
                                               name="o_sb")
                            nc.vector.tensor_tensor(o_sb, oraw, rr,
                                                    AluOpType.mult)
                            nc.scalar.dma_start(out=oT_s[b, h], in_=o_sb)
                        return tail

                    prev_tail = None
                    fill_items = []
                    tl = preload(0, 0)
                    for b in range(B_local):
                        for h in range(H):
                            # batch b-1's last oT store is emitted inside
                            # slot (b, 0)'s stream (deferred tail), so its
                            # projection reads may only be emitted from
                            # slot (b, 1) on
                            if do_proj and b > 0 and h == 1:
                                fill_items += make_proj_items(b - 1)
                            if h + 1 < H:
                                tl_next = preload(b, h + 1)
                            elif b + 1 < B_local:
                                tl_next = preload(b + 1, 0)
                            else:
                                tl_next = None
                            prev_tail = attention_slot(b, h, tl, fill_items,
                                                       prev_tail)
                            tl = tl_next
                    if prev_tail is not None:
                        prev_tail()
                    # drain remaining fillers + last batch's projection
                    if do_proj:
                        fill_items += make_proj_items(B_local - 1)
                    for it in fill_items:
                        while pending_drains:
                            pending_drains.pop(0)()
                        it()
                    while pending_drains:
                        pending_drains.pop(0)()
                    fill_items = []

    nc.compile()
    return nc


def preprocess(inputs, H=None):
    """Host-side parameter folding. Returns (arrays, meta)."""
    x = np.ascontiguousarray(
        np.asarray(inputs["x"], dtype=np.float32).astype(ml_dtypes.bfloat16))
    ln_g = np.asarray(inputs["ln_gamma"], dtype=np.float32)
    ln_b = np.asarray(inputs["ln_beta"], dtype=np.float32)
    qkv_w = np.asarray(inputs["qkv_w"], dtype=np.float32)
    qkv_b = np.asarray(inputs["qkv_b"], dtype=np.float32)
    proj_w = np.ascontiguousarray(
        np.asarray(inputs["proj_w"], dtype=np.float32).astype(ml_dtypes.bfloat16))
    proj_b = np.asarray(inputs["proj_b"], dtype=np.float32)
    ab = np.asarray(inputs["attention_biases"], dtype=np.float32)
    idx = np.asarray(inputs["bias_idxs"])

    B, N, DIM = x.shape
    if H is None:
        H = ab.shape[0]
    D = 128
    DH = H * D
    NT = N // 128
    NBLK = 2 * NT - 1
    assert qkv_w.shape == (DIM, 3 * DH)
    SCALE = float(D) ** -0.5

    W = qkv_w * ln_g[:, None]
    bfull = qkv_b + ln_b @ qkv_w
    Wq = W[:, :DH] * SCALE
    bq = bfull[:DH] * SCALE
    Wk = W[:, DH:2 * DH]
    bk = bfull[DH:2 * DH]
    Wv_flat = W[:, 2 * DH:].astype(ml_dtypes.bfloat16)
    VB = min(512, DH)
    Wv = np.ascontiguousarray(
        Wv_flat.reshape(DIM // 128, 128, DH // VB, VB).transpose(2, 1, 0, 3))
    bv = bfull[2 * DH:]
    wqk_flat = np.concatenate([Wq, Wk], axis=1).astype(ml_dtypes.bfloat16)
    KT, HC2 = DIM // 128, (2 * DH) // 128
    wqk = np.ascontiguousarray(
        wqk_flat.reshape(KT, 128, HC2, 128).transpose(2, 1, 0, 3))
    qb_qk = np.concatenate([bq, bk])

    # biasblk[h, p, (delta+NT-1)*128 + q'] = exp(ab[h, idx[i, j]]) where the
    # [N, N] bias matrix is block-Toeplitz over 128-token blocks: the value
    # depends only on delta = query_block - key_block.
    blk = np.empty((H, 128, NBLK * 128), dtype=np.float32)
    for delta in range(-(NT - 1), NT):
        jc0 = max(0, -delta)
        qb0 = jc0 + delta
        sub = idx[qb0 * 128:(qb0 + 1) * 128, jc0 * 128:(jc0 + 1) * 128]
        blk[:, :, (delta + NT - 1) * 128:(delta + NT) * 128] = \
            np.exp(ab[:, sub.T], dtype=np.float32)
    biasblk = np.ascontiguousarray(blk.astype(ml_dtypes.bfloat16))

    qkv_bias = bool(np.any(qb_qk != 0.0) or np.any(bv != 0.0))
    proj_bias = bool(np.any(proj_b != 0.0))

    arrays = dict(x=x, wqk=wqk, wv=Wv, wp=proj_w, biasblk=biasblk)
    if qkv_bias:
        arrays["qb_qk"] = np.ascontiguousarray(qb_qk)
        arrays["qb_v"] = np.ascontiguousarray(bv)
    if proj_bias:
        arrays["pb"] = np.ascontiguousarray(proj_b)
    meta = dict(B=B, N=N, DIM=DIM, H=H, qkv_bias=qkv_bias, proj_bias=proj_bias)
    return arrays, meta


_PROGRAM_CACHE = {}


def _get_program(key, **kw):
    if key not in _PROGRAM_CACHE:
        _PROGRAM_CACHE[key] = build_program(**kw)
    return _PROGRAM_CACHE[key]


def run(inputs, trace=False):
    """Run on the 8 NeuronCores. Returns (output, BassKernelResults)."""
    arrays, meta = preprocess(inputs)
    B, N, DIM, H = meta["B"], meta["N"], meta["DIM"], meta["H"]
    assert B % N_CORES == 0, f"batch {B} not divisible by {N_CORES} cores"
    B_local = B // N_CORES

    key = (B_local, N, DIM, H, meta["qkv_bias"], meta["proj_bias"])
    nc = _get_program(key, B_local=B_local, N=N, DIM=DIM, H=H,
                      qkv_bias=meta["qkv_bias"], proj_bias=meta["proj_bias"])

    shared = {k: v for k, v in arrays.items() if k != "x"}
    in_maps = []
    for c in range(N_CORES):
        m = dict(shared)
        m["x"] = np.ascontiguousarray(arrays["x"][c * B_local:(c + 1) * B_local])
        in_maps.append(m)

    try:
        res = run_bass_kernel_spmd(nc, in_maps, core_ids=list(range(N_CORES)),
                                   trace=trace)
    except ModuleNotFoundError:
        res = run_bass_kernel_spmd(nc, in_maps, core_ids=list(range(N_CORES)),
                                   trace=False)
    out = np.concatenate([res.results[c]["out"] for c in range(N_CORES)], axis=0)
    return out, res


def kernel(**inputs):
    out, _ = run(inputs, trace=False)
    return out
